# revision 30
# baseline (speedup 1.0000x reference)
"""MoE (top-2 of 8 experts, SwiGLU) Trainium2 kernel, expert-parallel over 8 cores.

Contract: kernel(**inputs) takes the FULL unsharded inputs
  x [2,2048,1024] f32, gate_w [8,1024] f32,
  w1 [8,2048,1024] f32, w2 [8,1024,2048] f32, w3 [8,2048,1024] f32
and returns the FULL output [2,2048,1024] f32.

Strategy (expert-parallel): routing (gate softmax + top-2) runs on host;
tokens are gathered per expert; core e runs the SwiGLU FFN of expert e over
its assigned tokens padded to capacity C (= N*TOPK/E rounded to chunks);
capacity-overflow tokens (<=64/expert) take the host FFN; the host
scatter-adds the two expert contributions per token.

Device kernel (per core), all matmul operands fp16 (same 1 cycle/row PE rate
as fp32r at these sizes, half the DMA/SBUF traffic, lower PE power):
  h1T = w1 @ xgT   [H, C]
  h3T = w3 @ xgT   [H, C]
  aT  = silu(h1T) * h3T      (ACT Silu psum->sbuf fp16, DVE mul)
  yT  = (w2 @ aT) * combine  [D, C]  (DVE mul on psum eviction, fp16 out)

All DRAM tensors use partition-major tile layouts so every DMA moves
2-4KB contiguous runs per partition (DMA packet rate, not bandwidth, was
the original bottleneck).  Dependency tracking is conservative per tile
(a read waits for ALL earlier writes to that tile), so gating granularity
comes from tile structure: one tile per (k, chunk) of xg and separate
w1/w3 tiles per h-block.  DMAs are issued in priority tiers matched to
first-need across the three DMA queues (sync/scalar hardware DGE +
gpsimd software DGE); everything not needed in the first two h-block
iterations is held behind gate DMAs/copies that depend on early act
evictions, because the fabric only sustains ~270GB/s and an eager
transfer steals bandwidth from the startup-critical ones.  The scalar
queue stays short: it is also the ACT engine, and each DMA dispatch
costs ~0.6-1.2us of engine time ahead of the first Silu eviction.
A burst of dummy matmuls at t=0 ramps the PE p-state (0.65->2.4GHz,
~3us of continuous busy needed) while the first DMAs are in flight;
stage A then runs gapless at the 1 col/cycle streaming floor.
"""

import math
import sys

import numpy as np

for _p in ("/opt/trn_rl_repo", "/opt/pypackages"):
    if _p not in sys.path:
        sys.path.append(_p)

import concourse.bass as bass  # noqa: E402
import concourse.tile as tile  # noqa: E402
from concourse import bacc, mybir  # noqa: E402
from concourse.bass_utils import run_bass_kernel_spmd  # noqa: E402

B, T, D, H, E, TOPK = 2, 2048, 1024, 2048, 8, 2
N = B * T
P = 128
KD = D // P   # 8  k-tiles over D
KH = H // P   # 16 k-tiles over H
HB = H // P   # 16 h blocks of 128 (M dim, stage A)
DB = D // P   # 8  d blocks of 128 (M dim, stage B)

F32 = mybir.dt.float32
F16 = mybir.dt.float16

# set by test.py to capture an NTFF profile; kernel() stores results here
TRACE = False
TRACE_ALL_CORES = False
LAST_RESULTS = None

_program_cache = {}

# CoreSim doesn't implement Silu; simcheck can override this to Sigmoid.
_ACT_FUNC = mybir.ActivationFunctionType.Silu

# dummy matmuls at t=0 to ramp the PE p-state while DMAs land
WARMUP_MM = 28

# Max tokens per expert handled on host when the count barely exceeds a
# 512 multiple (capacity-factor overflow).
OVERFLOW_MAX = 64


def _chunk_plan(cmax: int) -> list[int]:
    """Token-chunk sizes for the device capacity: each <=512 (PSUM bank),
    as equal as possible, 32-aligned, minimal total padding. If cmax is
    within OVERFLOW_MAX above a 512 multiple, use full 512 chunks and let
    the caller route the overflow tokens to the host FFN."""
    if cmax >= 512 and cmax - (cmax // 512) * 512 <= OVERFLOW_MAX:
        return [512] * (cmax // 512)
    n = max(1, math.ceil(cmax / 512))
    chunks = []
    rem = cmax
    for i in range(n):
        s = math.ceil(rem / (n - i) / 32) * 32
        s = min(max(s, 256), 512)
        chunks.append(s)
        rem -= s
    return chunks


def _host_ffn(x_rows, w1e, w2e, w3e, wts):
    """Exact host-side SwiGLU FFN for capacity-overflow tokens (<=64/expert)."""
    h1 = x_rows @ w1e.T
    h3 = x_rows @ w3e.T
    a = h1 / (1.0 + np.exp(-h1)) * h3
    return (a @ w2e.T) * wts[:, None]


def _build_program(chunks: list[int]):
    """Bass program for one core: expert FFN over C = sum(chunks) tokens."""
    C = sum(chunks)
    offs = [sum(chunks[:i]) for i in range(len(chunks))]
    tsls = [bass.ds(o, s) for o, s in zip(offs, chunks)]
    nt = len(chunks)

    nc = bacc.Bacc(
        "TRN2", target_bir_lowering=False, debug=False,
        enable_asserts=False, num_devices=8,
    )
    # partition-major layouts: per-partition runs are contiguous in DRAM
    xg_d = nc.dram_tensor("xg", [P, KD, C], F16, kind="ExternalInput").ap()
    w13_d = nc.dram_tensor("w13", [HB, P, 2, KD, P], F16,
                           kind="ExternalInput").ap()
    w2_d = nc.dram_tensor("w2a", [P, DB, KH, P], F16,
                          kind="ExternalInput").ap()
    scl_d = nc.dram_tensor("scale_b", [P, C], F32, kind="ExternalInput").ap()
    yT_d = nc.dram_tensor("yT", [DB, P, C], F16, kind="ExternalOutput").ap()

    # h-blocks with resident (pre-orchestrated) w1/w3 tiles; hb6+ stream
    # through a pool on the gpsimd software queue whose first dispatch is
    # held back (dummy dependency on hb1's eviction) so it cannot flood
    # the fabric during the startup-critical window.
    RES_HB = min(6, HB)

    with tile.TileContext(nc) as tc:
        with tc.tile_pool(name="resident", bufs=1) as res_pool, \
             tc.tile_pool(name="w13s", bufs=4) as w13_pool, \
             tc.tile_pool(name="ev", bufs=3) as ev_pool, \
             tc.tile_pool(name="psum", bufs=2, space="PSUM") as ps_pool:

            # PE p-state warmup: dummy matmuls on a zeroed tile while the
            # first DMAs are in flight (memset on gpsimd, which is
            # otherwise idle at boot).
            warm = res_pool.tile([P, 512], F16, tag="warm")
            nc.gpsimd.memset(warm[:], 0.0)
            pwarm = ps_pool.tile([P, 512], F32, tag="pwarm", bufs=1)
            for _ in range(WARMUP_MM):
                nc.tensor.matmul(pwarm[:], warm[:, 0:P], warm[:],
                                 start=True, stop=True)

            # Dependency tracking is conservative per tile (a read waits
            # for ALL earlier writes to that tile), so gating granularity
            # comes from tile structure: one tile per (k, chunk) of xg,
            # and separate w1/w3 tiles per h-block.
            xgs = [[res_pool.tile([P, chunks[t]], F16, tag=f"xg_{k}_{t}",
                                  name=f"xg_{k}_{t}")
                    for t in range(nt)] for k in range(KD)]
            act = res_pool.tile([P, KH, C], F16, tag="act")
            scl = res_pool.tile([P, C], F32, tag="scl")
            # w2 resident; transferred in 4 chained 1MB pieces so no DMA
            # ring is held for the ~28us a single 4MB transfer would take
            w2all = res_pool.tile([P, DB, KH, P], F16, tag="w2all")
            w1ts = {hb: res_pool.tile([P, KD, P], F16, tag=f"w1_{hb}",
                                      name=f"w1_{hb}")
                    for hb in range(RES_HB)}
            w3ts = {hb: res_pool.tile([P, KD, P], F16, tag=f"w3_{hb}",
                                      name=f"w3_{hb}")
                    for hb in range(RES_HB)}

            # ---- startup DMA orchestration ----------------------------
            # Priority tiers: the fabric sustains only ~270GB/s total, so
            # tier-0 (xg + hb0/hb1/hb2, ~3.8MB) runs alone; everything
            # else is held behind a gate DMA whose source is an act
            # region, so its dispatch (and the ring transfers queued
            # after it) waits for hb0's eviction.  The scalar engine is
            # also the ACT/eviction engine, so its dispatch list stays
            # short (each dispatch costs ~0.6-1.2us of engine time ahead
            # of the first Silu in program order).
            def xdma(q, k, t):
                q.dma_start(xgs[k][t][:], xg_d[:, k, tsls[t]])

            # scalar tier-0 (short list: this engine runs evictions):
            # k0/c0 first (smallest gate), w3t0, k1 c0, Silu-table
            # preload, hb1
            xdma(nc.scalar, 0, 0)
            nc.scalar.dma_start(w3ts[0][:], w13_d[0, :, 1])
            xdma(nc.scalar, 1, 0)
            dummy_act = res_pool.tile([P, 1], F16, tag="dummy_act")
            nc.scalar.activation(dummy_act[:], warm[:, 0:1], func=_ACT_FUNC)
            nc.scalar.dma_start(w1ts[1][:], w13_d[1, :, 0])
            nc.scalar.dma_start(w3ts[1][:], w13_d[1, :, 1])
            for t in range(1, nt):
                xdma(nc.scalar, 1, t)
                xdma(nc.scalar, 3, t)

            # sync tier-0: w1t0, even xg c0, even+k5/k7 xg c1, hb2
            nc.sync.dma_start(w1ts[0][:], w13_d[0, :, 0])
            xdma(nc.sync, 2, 0)
            xdma(nc.sync, 4, 0)
            xdma(nc.sync, 6, 0)
            for t in range(1, nt):
                for k in (0, 2, 4, 6, 5, 7):
                    xdma(nc.sync, k, t)
            if 2 < RES_HB:
                nc.sync.dma_start(w1ts[2][:], w13_d[2, :, 0])

            # gpsimd tier-0: k3/k5/k7 c0, w3t2, hb3
            for k in (3, 5, 7):
                xdma(nc.gpsimd, k, 0)
            if 2 < RES_HB:
                nc.gpsimd.dma_start(w3ts[2][:], w13_d[2, :, 1])
            if 3 < RES_HB:
                nc.gpsimd.dma_start(w1ts[3][:], w13_d[3, :, 0])
                nc.gpsimd.dma_start(w3ts[3][:], w13_d[3, :, 1])

            # ---- stage A: act[H, C] = silu(w1 @ xgT) * (w3 @ xgT) ----
            def a_iter(hb, t, w1t, w3t):
                tsl = tsls[t]
                ph1 = ps_pool.tile([P, chunks[t]], F32, tag="h1",
                                   bufs=3, name=f"ph1_{hb}_{t}")
                for k in range(KD):
                    nc.tensor.matmul(ph1[:], w1t[:, k, :], xgs[k][t][:],
                                     start=(k == 0), stop=(k == KD - 1))
                ph3 = ps_pool.tile([P, chunks[t]], F32, tag="h3",
                                   bufs=2, name=f"ph3_{hb}_{t}")
                for k in range(KD):
                    nc.tensor.matmul(ph3[:], w3t[:, k, :], xgs[k][t][:],
                                     start=(k == 0), stop=(k == KD - 1))
                asl = act[:, hb, tsl]
                nc.scalar.activation(asl, ph1[:], func=_ACT_FUNC)
                nc.vector.tensor_mul(asl, asl, ph3[:])

            # hb0/hb1 run chunk 0 first (their chunk-1 xg arrives one
            # iteration later), then their remaining chunks.
            a_iter(0, 0, w1ts[0], w3ts[0])
            if nt > 1:
                a_iter(1, 0, w1ts[1], w3ts[1])
            # sync tier-1, gated on hb1-c0's eviction (tier-0's tail is
            # still in flight before that): hb4, hb5, w2 pieces, scl
            gate_sb = res_pool.tile([P, 1], F16, tag="gate_sb")
            nc.sync.dma_start(gate_sb[:],
                              act[:, 1 if nt > 1 else 0, 0:1])
            for hb in (4, 5):
                if hb < RES_HB:
                    nc.sync.dma_start(w1ts[hb][:], w13_d[hb, :, 0])
                    nc.sync.dma_start(w3ts[hb][:], w13_d[hb, :, 1])
            for db in range(0, DB, 2):
                nc.sync.dma_start(w2all[:, db:db + 2], w2_d[:, db:db + 2])
            nc.sync.dma_start(scl[:], scl_d[:, :])
            if nt > 1:
                for t in range(1, nt):
                    a_iter(0, t, w1ts[0], w3ts[0])
                for t in range(1, nt):
                    a_iter(1, t, w1ts[1], w3ts[1])
            # gpsimd tier-2, gated on hb1's last eviction: stream hb6+
            gp_gate = res_pool.tile([P, 64], F16, tag="gp_gate")
            nc.gpsimd.tensor_copy(gp_gate[:], act[:, 1 if nt > 1 else 0,
                                                  0:64])
            for hb in range(1 if nt == 1 else 2, HB):
                if hb < RES_HB:
                    w1t, w3t = w1ts[hb], w3ts[hb]
                else:
                    w1t = w13_pool.tile([P, KD, P], F16, tag="w1s",
                                        name=f"w1s_{hb}")
                    nc.gpsimd.dma_start(w1t[:], w13_d[hb, :, 0])
                    w3t = w13_pool.tile([P, KD, P], F16, tag="w3s",
                                        name=f"w3s_{hb}")
                    nc.gpsimd.dma_start(w3t[:], w13_d[hb, :, 1])
                for t in range(nt):
                    a_iter(hb, t, w1t, w3t)

            # ---- stage B: yT[D, C] = (w2 @ act) * scale ----
            for d in range(DB):
                for t in range(nt):
                    # the very last group runs as two halves so the final
                    # eviction + output DMA after the last matmul is short
                    last = (d == DB - 1 and t == nt - 1)
                    h = chunks[t] // 2 if (last and chunks[t] % 64 == 0) \
                        else chunks[t]
                    for off in range(0, chunks[t], h):
                        tsl = bass.ds(offs[t] + off, h)
                        py = ps_pool.tile([P, h], F32, tag="y",
                                          name=f"py_{d}_{t}_{off}")
                        for k in range(KH):
                            nc.tensor.matmul(py[:], w2all[:, d, k, :],
                                             act[:, k, tsl],
                                             start=(k == 0),
                                             stop=(k == KH - 1))
                        ysb = ev_pool.tile([P, h], F16, tag="ysb",
                                           name=f"ysb_{d}_{t}_{off}")
                        nc.vector.tensor_mul(ysb[:], py[:], scl[:, tsl])
                        nc.scalar.dma_start(yT_d[d, :, tsl], ysb[:])

    nc.compile()
    return nc


def _route(flat, gate_w):
    """Host replica of the reference router. Returns top-2 expert ids and
    combine weights (top-2 of softmax, renormalized)."""
    logits = flat @ gate_w.T                                   # [N, E] f32
    m = logits.max(axis=1, keepdims=True)
    p = np.exp((logits - m).astype(np.float32))
    probs = p / p.sum(axis=1, keepdims=True)
    idx = np.argsort(-probs, axis=1, kind="stable")[:, :TOPK]  # [N, 2]
    top = np.take_along_axis(probs, idx, axis=1)               # [N, 2]
    wn = top / top.sum(axis=1, keepdims=True)
    return idx, wn


def kernel(x, gate_w, w1, w2, w3):
    global LAST_RESULTS
    x = np.asarray(x, np.float32)
    gate_w = np.asarray(gate_w, np.float32)
    w1 = np.asarray(w1, np.float32)
    w2 = np.asarray(w2, np.float32)
    w3 = np.asarray(w3, np.float32)

    flat = x.reshape(N, D)
    idx, wn = _route(flat, gate_w)

    sels, wsels = [], []
    for e in range(E):
        hit = idx == e                                         # [N, 2]
        sel = np.nonzero(hit.any(axis=1))[0]
        k = hit[sel, 1].astype(np.int64)                       # which top slot
        sels.append(sel)
        wsels.append(wn[sel, k])
    cmax = max(len(s) for s in sels)
    chunks = _chunk_plan(cmax)
    C = sum(chunks)

    in_maps = []
    for e in range(E):
        sel = sels[e][:C]                  # tokens beyond C go to _host_ffn
        n = len(sel)
        # xg[p, k, c] = x[sel[c], k*128+p]
        xg = np.zeros((P, KD, C), np.float16)
        xs = flat[sel].astype(np.float16)                 # [n, D]
        xg[:, :, :n] = xs.T.reshape(KD, P, n).transpose(1, 0, 2)
        # w13[hb, p, s, k, f] = w{1,3}[e][hb*128+f, k*128+p]
        t1 = w1[e].astype(np.float16).reshape(HB, P, KD, P).transpose(0, 3, 2, 1)
        t3 = w3[e].astype(np.float16).reshape(HB, P, KD, P).transpose(0, 3, 2, 1)
        w13 = np.ascontiguousarray(
            np.stack([t1, t3], axis=2))                   # [HB, P, 2, KD, P]
        # w2a[p, db, k, f] = w2[e][db*128+f, k*128+p]
        w2a = np.ascontiguousarray(
            w2[e].astype(np.float16).reshape(DB, P, KH, P)
            .transpose(3, 0, 2, 1))                       # [P, DB, KH, P]
        scale_b = np.zeros((P, C), np.float32)
        scale_b[:, :n] = wsels[e][:C][None, :]
        in_maps.append({
            "xg": xg,
            "w13": w13,
            "w2a": w2a,
            "scale_b": scale_b,
        })

    key = tuple(chunks)
    if key not in _program_cache:
        _program_cache[key] = _build_program(chunks)
    nc = _program_cache[key]

    res = run_bass_kernel_spmd(
        nc, in_maps, core_ids=list(range(E)),
        trace=TRACE,
        trace_cores=list(range(E)) if (TRACE and TRACE_ALL_CORES) else None,
    )
    LAST_RESULTS = res

    out = np.zeros((N, D), np.float32)
    for e in range(E):
        sel = sels[e][:C]
        # yT[db, p, c] -> [c, d]
        y = res.results[e]["yT"].astype(np.float32)
        y = y.transpose(2, 0, 1).reshape(C, D)
        out[sel] += y[:len(sel)]
        over = sels[e][C:]
        if len(over):
            out[over] += _host_ffn(flat[over], w1[e], w2[e], w3[e],
                                   wsels[e][C:])
    return out.reshape(B, T, D)
